# revision 3
# baseline (speedup 1.0000x reference)
"""Trainium2 Bass kernel for nn_Detect: 3-level detection head.

Math (per level, reference):
    k = conv1x1(x, k_w) + k_b          # [b, 3*512, h, w]
    v = conv1x1(x, v_w) + v_b
    kv = k * v  (flattened to [n, 512] per anchor)
    r[n, t, o] = sum_d kv[n,d] * q[t,d] * out_w[o,d] + out_b[o]
with q = target @ q_w.T + q_b.

Device strategy (8 cores, SPMD):
  - shard every level's h axis 8 ways (balanced); params replicated
  - W2T[d, t*5+o] = q[t,d]*out_w[o,d] computed on device once per core
  - conv GEMMs produce kv[d, pix] tiles; r = W2T.T @ kv -> [80, pix]
  - host does layout only: transposes/reshape/shard/unshard
"""
import sys

import numpy as np

try:
    import concourse.bacc as bacc  # noqa: F401
except Exception:  # pragma: no cover
    sys.path.insert(0, "/opt/trn_rl_repo")

import concourse.bacc as bacc
import concourse.bass as bass
import concourse.tile as tile
from concourse import mybir
from concourse.bass_utils import run_bass_kernel_spmd

N_CORES = 8
NA, HD, NO, NT, TD = 3, 512, 5, 16, 512
CH = [256, 512, 1024]
HW = [64, 32, 16]
BS = 2
HS = [h // N_CORES for h in HW]          # per-core h rows: [8, 4, 2]
P = [BS * hs * w for hs, w in zip(HS, HW)]  # per-core pixels: [1024, 256, 64]
CCH = [c // 128 for c in CH]             # channel chunks: [2, 4, 8]
NTJ = NT * NO                            # 80
NB = 512                                 # matmul free-dim block

# matmul input dtype: float32 (exact, 4 cyc/row), float32r (fast fp32 mode),
# or bfloat16 (fast, lower precision). All accumulation stays fp32.
MM_DT = mybir.dt.float32r

_STATE = {}


def _build():
    f32 = mybir.dt.float32
    mmd = MM_DT
    nc = bacc.Bacc("TRN2", target_bir_lowering=False, debug=False,
                   num_devices=N_CORES)

    xs = [nc.dram_tensor(f"x{l}", [128, CCH[l] * P[l]], mmd,
                         kind="ExternalInput") for l in range(3)]
    ws = [nc.dram_tensor(f"w{l}", [128, CCH[l] * 2 * 1536], mmd,
                         kind="ExternalInput") for l in range(3)]
    qw = nc.dram_tensor("qw", [128, 4 * TD], mmd, kind="ExternalInput")
    tgt = nc.dram_tensor("tgt", [128, 4 * NT], mmd, kind="ExternalInput")
    par = nc.dram_tensor("par", [128, 97], f32, kind="ExternalInput")
    rs = [nc.dram_tensor(f"r{l}", [NTJ, NA * P[l]], f32,
                         kind="ExternalOutput") for l in range(3)]

    with tile.TileContext(nc) as tc:
        with (
            tc.tile_pool(name="const", bufs=1) as cpool,
            tc.tile_pool(name="x", bufs=2) as xpool,
            tc.tile_pool(name="w", bufs=8) as wpool,
            tc.tile_pool(name="vev", bufs=4) as vpool,
            tc.tile_pool(name="kv", bufs=4) as kvpool,
            tc.tile_pool(name="rout", bufs=3) as rpool,
            tc.tile_pool(name="ps", bufs=4, space="PSUM") as pspool,
            tc.tile_pool(name="psr", bufs=2, space="PSUM") as psrpool,
            tc.tile_pool(name="psq", bufs=1, space="PSUM") as psqpool,
        ):
            # ---- constants / params ----
            par_sb = cpool.tile([128, 97], f32, tag="par")
            nc.sync.dma_start(par_sb[:], par.ap())
            qw_sb = cpool.tile([128, 4 * TD], mmd, tag="qw")
            nc.sync.dma_start(qw_sb[:], qw.ap())
            tgt_sb = cpool.tile([128, 4 * NT], mmd, tag="tgt")
            nc.sync.dma_start(tgt_sb[:], tgt.ap())

            # ---- q = target @ q_w.T + q_b, computed as qT [512(hd), 16] ----
            qT_sb = cpool.tile([128, 4 * NT], f32, tag="qT")
            for m in range(4):
                psq = psqpool.tile([128, NT], f32)
                for cc in range(4):
                    nc.tensor.matmul(
                        psq[:],
                        qw_sb[:, cc * TD + m * 128: cc * TD + (m + 1) * 128],
                        tgt_sb[:, cc * NT:(cc + 1) * NT],
                        start=(cc == 0), stop=(cc == 3),
                    )
                nc.scalar.activation(
                    qT_sb[:, m * NT:(m + 1) * NT], psq[:],
                    mybir.ActivationFunctionType.Identity,
                    bias=par_sb[:, 72 + m:73 + m],
                )

            # ---- W2T[d, t*5+j] = qT[d, t] * out_wT[d, j] ----
            w2_sb = cpool.tile([128, 4 * NTJ], mmd, tag="w2")
            w2_4d = w2_sb[:].rearrange("p (m t j) -> p m t j", m=4, t=NT, j=NO)
            for m in range(4):
                for j in range(NO):
                    nc.vector.tensor_scalar_mul(
                        w2_4d[:, m, :, j],
                        qT_sb[:, m * NT:(m + 1) * NT],
                        par_sb[:, 76 + m * NO + j:77 + m * NO + j],
                    )

            # ---- levels ----
            for l in range(3):
                cch, pl = CCH[l], P[l]
                x_sb = xpool.tile([128, cch * pl], mmd, tag="x")
                nc.sync.dma_start(x_sb[:], xs[l].ap())

                w_sb = [wpool.tile([128, 2 * 1536], mmd, tag="w",
                                   name=f"w_l{l}c{cc}")
                        for cc in range(cch)]
                for cc in range(cch):
                    nc.sync.dma_start(
                        w_sb[cc][:], ws[l].ap()[:, cc * 3072:(cc + 1) * 3072])

                kv_sb = [kvpool.tile([128, NA * pl], mmd, tag="kv",
                                     name=f"kv_l{l}d{d}")
                         for d in range(4)]

                npb = (pl + NB - 1) // NB
                for ot in range(12):            # otile = a*4 + dchunk
                    a, dchunk = divmod(ot, 4)
                    for pb in range(npb):
                        pw = min(NB, pl - pb * NB)
                        psk = pspool.tile([128, NB], f32, tag="psc")
                        psv = pspool.tile([128, NB], f32, tag="psc")
                        for cc in range(cch):
                            nc.tensor.matmul(
                                psk[:, :pw],
                                w_sb[cc][:, ot * 128:(ot + 1) * 128],
                                x_sb[:, cc * pl + pb * NB:
                                     cc * pl + pb * NB + pw],
                                start=(cc == 0), stop=(cc == cch - 1),
                            )
                        for cc in range(cch):
                            nc.tensor.matmul(
                                psv[:, :pw],
                                w_sb[cc][:, 1536 + ot * 128:
                                         1536 + (ot + 1) * 128],
                                x_sb[:, cc * pl + pb * NB:
                                     cc * pl + pb * NB + pw],
                                start=(cc == 0), stop=(cc == cch - 1),
                            )
                        v_sb = vpool.tile([128, NB], f32, tag="vev")
                        nc.scalar.activation(
                            v_sb[:, :pw], psv[:, :pw],
                            mybir.ActivationFunctionType.Identity,
                            bias=par_sb[:, l * 24 + 12 + ot:l * 24 + 13 + ot],
                        )
                        # kv = (k + kb) * v
                        nc.vector.scalar_tensor_tensor(
                            kv_sb[dchunk][:, a * pl + pb * NB:
                                          a * pl + pb * NB + pw],
                            psk[:, :pw],
                            par_sb[:, l * 24 + ot:l * 24 + 1 + ot],
                            v_sb[:, :pw],
                            op0=mybir.AluOpType.add,
                            op1=mybir.AluOpType.mult,
                        )

                # r[tj, pix] = sum_d W2T[d, tj] * kv[d, pix]  (+ out_b)
                napl = NA * pl
                nnb = (napl + NB - 1) // NB
                for nb_i in range(nnb):
                    nw = min(NB, napl - nb_i * NB)
                    psr = psrpool.tile([NTJ, NB], f32, tag="psr")
                    for dchunk in range(4):
                        nc.tensor.matmul(
                            psr[:, :nw],
                            w2_sb[:, dchunk * NTJ:(dchunk + 1) * NTJ],
                            kv_sb[dchunk][:, nb_i * NB:nb_i * NB + nw],
                            start=(dchunk == 0), stop=(dchunk == 3),
                        )
                    r_sb = rpool.tile([NTJ, NB], f32, tag="rout")
                    nc.scalar.activation(
                        r_sb[:, :nw], psr[:, :nw],
                        mybir.ActivationFunctionType.Identity,
                        bias=par_sb[:NTJ, 96:97],
                    )
                    nc.sync.dma_start(
                        rs[l].ap()[:, nb_i * NB:nb_i * NB + nw], r_sb[:, :nw])

    nc.compile()
    return nc


def _chunk128(arr):
    """[C, F] -> [128, (C//128)*F] with chunk-major columns."""
    c, f = arr.shape
    return np.ascontiguousarray(
        arr.reshape(c // 128, 128, f).transpose(1, 0, 2).reshape(128, -1))


def _prep(inputs):
    """Host-side layout prep. Returns per-core input maps."""
    from concourse import mybir as _mb
    mm_np = _mb.dt.np(MM_DT)
    f = lambda k: np.asarray(inputs[k], dtype=np.float32)

    # shared (replicated) tensors
    shared = {}
    for l in range(3):
        kwT = f(f"k_w{l}").T                      # [c, 1536]
        vwT = f(f"v_w{l}").T
        cch = CCH[l]
        kw = kwT.reshape(cch, 128, 1536)
        vw = vwT.reshape(cch, 128, 1536)
        w = np.concatenate([kw, vw], axis=2)      # [cch, 128, 3072]
        shared[f"w{l}"] = np.ascontiguousarray(
            w.transpose(1, 0, 2).reshape(128, -1)).astype(mm_np)
    shared["qw"] = _chunk128(f("q_w").T).astype(mm_np)    # [128, 4*512]
    shared["tgt"] = _chunk128(f("target").T).astype(mm_np)  # [128, 4*16]

    par = np.zeros((128, 97), np.float32)
    for l in range(3):
        par[:, l * 24:l * 24 + 12] = f(f"k_b{l}").reshape(12, 128).T
        par[:, l * 24 + 12:l * 24 + 24] = f(f"v_b{l}").reshape(12, 128).T
    par[:, 72:76] = f("q_b").reshape(4, 128).T
    par[:, 76:96] = _chunk128(f("out_w").T)       # out_wT [512,5] -> [128,20]
    par[:NTJ, 96] = np.tile(f("out_b"), NT)
    shared["par"] = par

    in_maps = []
    for core in range(N_CORES):
        m = dict(shared)
        for l in range(3):
            hs, w_ = HS[l], HW[l]
            x = f(f"x{l}")[:, :, core * hs:(core + 1) * hs, :]  # [2,c,hs,w]
            xt = np.ascontiguousarray(x.transpose(1, 0, 2, 3)).reshape(CH[l], -1)
            m[f"x{l}"] = _chunk128(xt).astype(mm_np)
        in_maps.append(m)
    return in_maps


def _assemble(results):
    """Per-core [80, 3*P] outputs -> tuple of 3 full [2,16,3,h,w,5] arrays."""
    outs = []
    for l in range(3):
        hs, w_ = HS[l], HW[l]
        parts = []
        for core in range(N_CORES):
            r = results[core][f"r{l}"].reshape(NT, NO, NA, BS, hs, w_)
            parts.append(r.transpose(3, 0, 2, 4, 5, 1))  # [b,t,a,hs,w,j]
        outs.append(np.ascontiguousarray(np.concatenate(parts, axis=3)))
    return tuple(outs)


def _get_nc():
    if "nc" not in _STATE:
        _STATE["nc"] = _build()
    return _STATE["nc"]


def _run(inputs, **kw):
    nc = _get_nc()
    in_maps = _prep(inputs)
    res = run_bass_kernel_spmd(nc, in_maps, list(range(N_CORES)), **kw)
    return res


def kernel(**inputs):
    res = _run(inputs)
    return _assemble(res.results)


# revision 4
# speedup vs baseline: 1.3014x; 1.3014x over previous
"""Trainium2 Bass kernel for nn_Detect: 3-level detection head.

Math (per level, reference):
    k = conv1x1(x, k_w) + k_b          # [b, 3*512, h, w]
    v = conv1x1(x, v_w) + v_b
    kv = k * v  (flattened to [n, 512] per anchor)
    r[n, t, o] = sum_d kv[n,d] * q[t,d] * out_w[o,d] + out_b[o]
with q = target @ q_w.T + q_b.

Device strategy (8 cores, SPMD):
  - L0/L1: shard the h axis 8 ways (balanced); params replicated.
  - L2 (weight-heavy, few pixels): shard by (out-channel-tile, pixel-half)
    tasks -- 24 tasks, 3 per core, full fp32r matmul rate (N=256); the
    d-contraction of the second GEMM is then partial per core, so cores
    emit r2 partials that the host scatter-adds (bias out_b/4 per partial).
  - W2T[d, t*5+o] = q[t,d]*out_w[o,d] computed on device once per core.
  - conv GEMMs produce kv[d, pix] tiles; r = W2T.T @ kv -> [80, pix].
  - host does layout only: transposes/reshape/shard/unshard/partial-sum.
"""
import sys

import numpy as np

try:
    import concourse.bacc as bacc  # noqa: F401
except Exception:  # pragma: no cover
    sys.path.insert(0, "/opt/trn_rl_repo")

import concourse.bacc as bacc
import concourse.bass as bass  # noqa: F401
import concourse.tile as tile
from concourse import mybir
from concourse.bass_utils import run_bass_kernel_spmd

N_CORES = 8
NA, HD, NO, NT, TD = 3, 512, 5, 16, 512
CH = [256, 512, 1024]
HW = [64, 32, 16]
BS = 2
HS = [h // N_CORES for h in HW]             # per-core h rows (L0/L1 use)
P = [BS * hs * w for hs, w in zip(HS, HW)]  # per-core pixels [1024, 256, 64]
CCH = [c // 128 for c in CH]                # channel chunks [2, 4, 8]
NTJ = NT * NO                               # 80
NB = 512                                    # matmul free-dim block

# L2 task sharding: 24 tasks = (otile 0..11) x (pixel half 0..1), 3 per core
P2F = BS * HW[2] * HW[2]                    # 512 full L2 pixels
L2H = P2F // 2                              # 256 per half
L2T = 3                                     # tasks per core
L2TASKS = [(ot, half) for ot in range(12) for half in range(2)]

# par column layout (per-core)
PC_KB = [0, 24]            # L0/L1 k-bias base (12 cols each)
PC_VB = [12, 36]           # L0/L1 v-bias base
PC_KB2, PC_VB2 = 48, 51    # L2 per-task k/v bias (3 each)
PC_QB = 54                 # q_b chunks (4)
PC_OW = 58                 # out_w chunks, m-major (20)
PC_OB = 78                 # out_b tiled [80]
PC_OB4 = 79                # out_b/4 tiled (L2 partials)
PC_SEL = 80                # qT dchunk-selection one-hots (3 tasks x 4)
PC_OWT = 92                # L2 per-task out_w chunk (3 tasks x 5)
PC_N = 107

# matmul input dtype: float32 (exact, 4 cyc/row), float32r (fast fp32 mode),
# or bfloat16 (fast, lower precision). All accumulation stays fp32.
MM_DT = mybir.dt.float32r

_STATE = {}


def _build():
    f32 = mybir.dt.float32
    mmd = MM_DT
    act_id = mybir.ActivationFunctionType.Identity
    nc = bacc.Bacc("TRN2", target_bir_lowering=False, debug=False,
                   num_devices=N_CORES)

    xs = [nc.dram_tensor(f"x{l}", [128, CCH[l] * P[l]], mmd,
                         kind="ExternalInput") for l in range(2)]
    ws = [nc.dram_tensor(f"w{l}", [128, CCH[l] * 2 * 1536], mmd,
                         kind="ExternalInput") for l in range(2)]
    x2 = nc.dram_tensor("x2", [128, L2T * 8 * L2H], mmd,
                        kind="ExternalInput")
    w2t = nc.dram_tensor("w2t", [128, L2T * 2 * 8 * 128], mmd,
                         kind="ExternalInput")
    qw = nc.dram_tensor("qw", [128, 4 * TD], mmd, kind="ExternalInput")
    tgt = nc.dram_tensor("tgt", [128, 4 * NT], mmd, kind="ExternalInput")
    par = nc.dram_tensor("par", [128, PC_N], f32, kind="ExternalInput")
    rs = [nc.dram_tensor(f"r{l}", [NTJ, NA * P[l]], f32,
                         kind="ExternalOutput") for l in range(2)]
    r2 = nc.dram_tensor("r2", [NTJ, L2T * L2H], f32, kind="ExternalOutput")

    with tile.TileContext(nc) as tc:
        with (
            tc.tile_pool(name="const", bufs=1) as cpool,
            tc.tile_pool(name="x", bufs=2) as xpool,
            tc.tile_pool(name="x2p", bufs=1) as x2pool,
            tc.tile_pool(name="w", bufs=4) as wpool,
            tc.tile_pool(name="w2tp", bufs=1) as w2tpool,
            tc.tile_pool(name="vev", bufs=4) as vpool,
            tc.tile_pool(name="kv", bufs=4) as kvpool,
            tc.tile_pool(name="kv2", bufs=3) as kv2pool,
            tc.tile_pool(name="rout", bufs=3) as rpool,
            tc.tile_pool(name="ps", bufs=4, space="PSUM") as pspool,
            tc.tile_pool(name="psr", bufs=2, space="PSUM") as psrpool,
            tc.tile_pool(name="psq", bufs=1, space="PSUM") as psqpool,
        ):
            # ---- constants / params ----
            par_sb = cpool.tile([128, PC_N], f32, tag="par")
            nc.sync.dma_start(par_sb[:], par.ap())
            qw_sb = cpool.tile([128, 4 * TD], mmd, tag="qw")
            nc.sync.dma_start(qw_sb[:], qw.ap())
            tgt_sb = cpool.tile([128, 4 * NT], mmd, tag="tgt")
            nc.sync.dma_start(tgt_sb[:], tgt.ap())

            # ---- q = target @ q_w.T + q_b, computed as qT [512(hd), 16] ----
            qT_sb = cpool.tile([128, 4 * NT], f32, tag="qT")
            for m in range(4):
                psq = psqpool.tile([128, NT], f32)
                for cc in range(4):
                    nc.tensor.matmul(
                        psq[:],
                        qw_sb[:, cc * TD + m * 128: cc * TD + (m + 1) * 128],
                        tgt_sb[:, cc * NT:(cc + 1) * NT],
                        start=(cc == 0), stop=(cc == 3),
                    )
                nc.scalar.activation(
                    qT_sb[:, m * NT:(m + 1) * NT], psq[:], act_id,
                    bias=par_sb[:, PC_QB + m:PC_QB + m + 1],
                )

            # ---- W2T[d, t*5+j] = qT[d, t] * out_wT[d, j] ----
            w2_sb = cpool.tile([128, 4 * NTJ], mmd, tag="w2")
            w2_4d = w2_sb[:].rearrange("p (m t j) -> p m t j", m=4, t=NT, j=NO)
            for m in range(4):
                for j in range(NO):
                    nc.vector.tensor_scalar_mul(
                        w2_4d[:, m, :, j],
                        qT_sb[:, m * NT:(m + 1) * NT],
                        par_sb[:, PC_OW + m * NO + j:PC_OW + m * NO + j + 1],
                    )

            # ---- per-task qT selection + task W2T (L2 partials) ----
            qsel_sb = cpool.tile([128, L2T * NT], f32, tag="qsel")
            for t in range(L2T):
                o = t * NT
                nc.vector.tensor_scalar_mul(
                    qsel_sb[:, o:o + NT], qT_sb[:, 0:NT],
                    par_sb[:, PC_SEL + t * 4:PC_SEL + t * 4 + 1])
                for m in range(1, 4):
                    nc.vector.scalar_tensor_tensor(
                        qsel_sb[:, o:o + NT],
                        qT_sb[:, m * NT:(m + 1) * NT],
                        par_sb[:, PC_SEL + t * 4 + m:PC_SEL + t * 4 + m + 1],
                        qsel_sb[:, o:o + NT],
                        op0=mybir.AluOpType.mult,
                        op1=mybir.AluOpType.add,
                    )
            w2task_sb = cpool.tile([128, L2T * NTJ], mmd, tag="w2task")
            w2t_4d = w2task_sb[:].rearrange("p (t b j) -> p t b j",
                                            t=L2T, b=NT, j=NO)
            for t in range(L2T):
                for j in range(NO):
                    nc.vector.tensor_scalar_mul(
                        w2t_4d[:, t, :, j],
                        qsel_sb[:, t * NT:(t + 1) * NT],
                        par_sb[:, PC_OWT + t * NO + j:PC_OWT + t * NO + j + 1],
                    )

            # ---- levels 0/1: pixel-sharded ----
            for l in range(2):
                cch, pl = CCH[l], P[l]
                x_sb = xpool.tile([128, cch * pl], mmd, tag="x")
                nc.sync.dma_start(x_sb[:], xs[l].ap())

                w_sb = [wpool.tile([128, 2 * 1536], mmd, tag="w",
                                   name=f"w_l{l}c{cc}")
                        for cc in range(cch)]
                for cc in range(cch):
                    nc.sync.dma_start(
                        w_sb[cc][:], ws[l].ap()[:, cc * 3072:(cc + 1) * 3072])

                kv_sb = [kvpool.tile([128, NA * pl], mmd, tag="kv",
                                     name=f"kv_l{l}d{d}")
                         for d in range(4)]

                npb = (pl + NB - 1) // NB
                for ot in range(12):            # otile = a*4 + dchunk
                    a, dchunk = divmod(ot, 4)
                    for pb in range(npb):
                        pw = min(NB, pl - pb * NB)
                        psk = pspool.tile([128, NB], f32, tag="psc")
                        psv = pspool.tile([128, NB], f32, tag="psc")
                        for cc in range(cch):
                            nc.tensor.matmul(
                                psk[:, :pw],
                                w_sb[cc][:, ot * 128:(ot + 1) * 128],
                                x_sb[:, cc * pl + pb * NB:
                                     cc * pl + pb * NB + pw],
                                start=(cc == 0), stop=(cc == cch - 1),
                            )
                        for cc in range(cch):
                            nc.tensor.matmul(
                                psv[:, :pw],
                                w_sb[cc][:, 1536 + ot * 128:
                                         1536 + (ot + 1) * 128],
                                x_sb[:, cc * pl + pb * NB:
                                     cc * pl + pb * NB + pw],
                                start=(cc == 0), stop=(cc == cch - 1),
                            )
                        v_sb = vpool.tile([128, NB], f32, tag="vev")
                        nc.scalar.activation(
                            v_sb[:, :pw], psv[:, :pw], act_id,
                            bias=par_sb[:, PC_VB[l] + ot:PC_VB[l] + ot + 1],
                        )
                        # kv = (k + kb) * v
                        nc.vector.scalar_tensor_tensor(
                            kv_sb[dchunk][:, a * pl + pb * NB:
                                          a * pl + pb * NB + pw],
                            psk[:, :pw],
                            par_sb[:, PC_KB[l] + ot:PC_KB[l] + ot + 1],
                            v_sb[:, :pw],
                            op0=mybir.AluOpType.add,
                            op1=mybir.AluOpType.mult,
                        )

                # r[tj, pix] = sum_d W2T[d, tj] * kv[d, pix]  (+ out_b)
                napl = NA * pl
                nnb = (napl + NB - 1) // NB
                for nb_i in range(nnb):
                    nw = min(NB, napl - nb_i * NB)
                    psr = psrpool.tile([NTJ, NB], f32, tag="psr")
                    for dchunk in range(4):
                        nc.tensor.matmul(
                            psr[:, :nw],
                            w2_sb[:, dchunk * NTJ:(dchunk + 1) * NTJ],
                            kv_sb[dchunk][:, nb_i * NB:nb_i * NB + nw],
                            start=(dchunk == 0), stop=(dchunk == 3),
                        )
                    r_sb = rpool.tile([NTJ, NB], f32, tag="rout")
                    nc.scalar.activation(
                        r_sb[:, :nw], psr[:, :nw], act_id,
                        bias=par_sb[:NTJ, PC_OB:PC_OB + 1],
                    )
                    nc.sync.dma_start(
                        rs[l].ap()[:, nb_i * NB:nb_i * NB + nw], r_sb[:, :nw])

            # ---- level 2: (otile, pixel-half) task shard ----
            x2_sb = x2pool.tile([128, L2T * 8 * L2H], mmd, tag="x2")
            nc.sync.dma_start(x2_sb[:], x2.ap())
            w2t_sb = w2tpool.tile([128, L2T * 2 * 8 * 128], mmd, tag="w2t")
            nc.sync.dma_start(w2t_sb[:], w2t.ap())

            for t in range(L2T):
                psk = pspool.tile([128, NB], f32, tag="psc")
                psv = pspool.tile([128, NB], f32, tag="psc")
                kb = (t * 2 + 0) * 8 * 128
                vb = (t * 2 + 1) * 8 * 128
                for cc in range(8):
                    nc.tensor.matmul(
                        psk[:, :L2H],
                        w2t_sb[:, kb + cc * 128:kb + (cc + 1) * 128],
                        x2_sb[:, (t * 8 + cc) * L2H:(t * 8 + cc + 1) * L2H],
                        start=(cc == 0), stop=(cc == 7),
                    )
                for cc in range(8):
                    nc.tensor.matmul(
                        psv[:, :L2H],
                        w2t_sb[:, vb + cc * 128:vb + (cc + 1) * 128],
                        x2_sb[:, (t * 8 + cc) * L2H:(t * 8 + cc + 1) * L2H],
                        start=(cc == 0), stop=(cc == 7),
                    )
                v_sb = vpool.tile([128, NB], f32, tag="vev")
                nc.scalar.activation(
                    v_sb[:, :L2H], psv[:, :L2H], act_id,
                    bias=par_sb[:, PC_VB2 + t:PC_VB2 + t + 1],
                )
                kv_t = kv2pool.tile([128, L2H], mmd, tag="kv2")
                nc.vector.scalar_tensor_tensor(
                    kv_t[:], psk[:, :L2H],
                    par_sb[:, PC_KB2 + t:PC_KB2 + t + 1],
                    v_sb[:, :L2H],
                    op0=mybir.AluOpType.add,
                    op1=mybir.AluOpType.mult,
                )
                psr = psrpool.tile([NTJ, NB], f32, tag="psr")
                nc.tensor.matmul(
                    psr[:, :L2H],
                    w2task_sb[:, t * NTJ:(t + 1) * NTJ],
                    kv_t[:],
                    start=True, stop=True,
                )
                r_sb = rpool.tile([NTJ, NB], f32, tag="rout")
                nc.scalar.activation(
                    r_sb[:, :L2H], psr[:, :L2H], act_id,
                    bias=par_sb[:NTJ, PC_OB4:PC_OB4 + 1],
                )
                nc.sync.dma_start(
                    r2.ap()[:, t * L2H:(t + 1) * L2H], r_sb[:, :L2H])

    nc.compile()
    return nc


def _chunk128(arr):
    """[C, F] -> [128, (C//128)*F] with chunk-major columns."""
    c, f = arr.shape
    return np.ascontiguousarray(
        arr.reshape(c // 128, 128, f).transpose(1, 0, 2).reshape(128, -1))


def _prep(inputs):
    """Host-side layout prep. Returns per-core input maps."""
    mm_np = mybir.dt.np(MM_DT)
    f = lambda k: np.asarray(inputs[k], dtype=np.float32)

    # shared (replicated) tensors
    shared = {}
    for l in range(2):
        kwT = f(f"k_w{l}").T                      # [c, 1536]
        vwT = f(f"v_w{l}").T
        cch = CCH[l]
        kw = kwT.reshape(cch, 128, 1536)
        vw = vwT.reshape(cch, 128, 1536)
        w = np.concatenate([kw, vw], axis=2)      # [cch, 128, 3072]
        shared[f"w{l}"] = np.ascontiguousarray(
            w.transpose(1, 0, 2).reshape(128, -1)).astype(mm_np)
    shared["qw"] = _chunk128(f("q_w").T).astype(mm_np)
    shared["tgt"] = _chunk128(f("target").T).astype(mm_np)

    kwT2 = f("k_w2").T                            # [1024, 1536]
    vwT2 = f("v_w2").T
    kb2 = f("k_b2").reshape(12, 128)
    vb2 = f("v_b2").reshape(12, 128)
    ow128 = _chunk128(f("out_w").T)               # [128, 20] m-major

    par_base = np.zeros((128, PC_N), np.float32)
    for l in range(2):
        par_base[:, PC_KB[l]:PC_KB[l] + 12] = f(f"k_b{l}").reshape(12, 128).T
        par_base[:, PC_VB[l]:PC_VB[l] + 12] = f(f"v_b{l}").reshape(12, 128).T
    par_base[:, PC_QB:PC_QB + 4] = f("q_b").reshape(4, 128).T
    par_base[:, PC_OW:PC_OW + 20] = ow128
    par_base[:NTJ, PC_OB] = np.tile(f("out_b"), NT)
    par_base[:NTJ, PC_OB4] = np.tile(f("out_b"), NT) / 4.0

    # L2 x, full pixels, chunk-major: [128, 8, 512]
    xt2 = np.ascontiguousarray(
        f("x2").transpose(1, 0, 2, 3)).reshape(CH[2], -1)
    x2c = _chunk128(xt2).reshape(128, 8, P2F)

    in_maps = []
    for core in range(N_CORES):
        m = dict(shared)
        for l in range(2):
            hs = HS[l]
            x = f(f"x{l}")[:, :, core * hs:(core + 1) * hs, :]  # [2,c,hs,w]
            xt = np.ascontiguousarray(x.transpose(1, 0, 2, 3)).reshape(CH[l], -1)
            m[f"x{l}"] = _chunk128(xt).astype(mm_np)

        tasks = L2TASKS[core * L2T:(core + 1) * L2T]
        par = par_base.copy()
        wcols = np.zeros((128, L2T * 2 * 8 * 128), np.float32)
        x2cols = np.zeros((128, L2T * 8 * L2H), np.float32)
        for t, (ot, half) in enumerate(tasks):
            dchunk = ot % 4
            par[:, PC_KB2 + t] = kb2[ot]
            par[:, PC_VB2 + t] = vb2[ot]
            par[:, PC_SEL + t * 4 + dchunk] = 1.0
            par[:, PC_OWT + t * NO:PC_OWT + (t + 1) * NO] = \
                ow128[:, dchunk * NO:(dchunk + 1) * NO]
            wcols[:, (t * 2) * 1024:(t * 2 + 1) * 1024] = \
                _chunk128(kwT2[:, ot * 128:(ot + 1) * 128])
            wcols[:, (t * 2 + 1) * 1024:(t * 2 + 2) * 1024] = \
                _chunk128(vwT2[:, ot * 128:(ot + 1) * 128])
            x2cols[:, t * 8 * L2H:(t + 1) * 8 * L2H] = \
                x2c[:, :, half * L2H:(half + 1) * L2H].reshape(128, -1)
        m["par"] = par
        m["w2t"] = wcols.astype(mm_np)
        m["x2"] = x2cols.astype(mm_np)
        in_maps.append(m)
    return in_maps


def _assemble(results):
    """Per-core outputs -> tuple of 3 full [2,16,3,h,w,5] arrays."""
    outs = []
    for l in range(2):
        hs, w_ = HS[l], HW[l]
        parts = []
        for core in range(N_CORES):
            r = results[core][f"r{l}"].reshape(NT, NO, NA, BS, hs, w_)
            parts.append(r.transpose(3, 0, 2, 4, 5, 1))  # [b,t,a,hs,w,j]
        outs.append(np.ascontiguousarray(np.concatenate(parts, axis=3)))

    full2 = np.zeros((NTJ, NA, P2F), np.float64)
    for core in range(N_CORES):
        tasks = L2TASKS[core * L2T:(core + 1) * L2T]
        rc = results[core]["r2"]
        for t, (ot, half) in enumerate(tasks):
            a = ot // 4
            full2[:, a, half * L2H:(half + 1) * L2H] += \
                rc[:, t * L2H:(t + 1) * L2H]
    r2 = full2.astype(np.float32).reshape(NT, NO, NA, BS, HW[2], HW[2])
    outs.append(np.ascontiguousarray(r2.transpose(3, 0, 2, 4, 5, 1)))
    return tuple(outs)


def _get_nc():
    if "nc" not in _STATE:
        _STATE["nc"] = _build()
    return _STATE["nc"]


def _run(inputs, **kw):
    nc = _get_nc()
    in_maps = _prep(inputs)
    res = run_bass_kernel_spmd(nc, in_maps, list(range(N_CORES)), **kw)
    return res


def kernel(**inputs):
    res = _run(inputs)
    return _assemble(res.results)
